# revision 7
# baseline (speedup 1.0000x reference)
"""AdaBlock (binarized double-conv residual block) Trainium2 kernel.

Strategy
--------
Data-parallel over batch: 16 images across 8 NeuronCores (2 images/core), no
collectives.  The binarized convs are exact +-1 matmuls: a 3x3 conv is 9
shifted [Cin x spatial] matmuls accumulated in PSUM.  fp8 with
`perf_mode=DoubleRow` packs both 128-channel cin halves into one K=256
matmul (~90 ns / N=462 matmul measured on HW).  Sign activations are
computed as (x >= -b) - 0.5 on the VectorE (values +-0.5, exact in fp8; the
factor 2 is folded into the per-out-channel conv scales), intermediates are
fp16 to hit the DVE 2x/4x perf modes, PSUM accumulation stays fp32 so conv
sums are exact; overall mean rel err vs the fp32 reference ~2e-3.

Spatial layout: sign activations live in a zero-ring-padded 66x66 grid per
cin half (flat, half-stride 4368 so the DoubleRow rhs AP is [p, 2, N]).
Conv output is tiled over 7 padded rows (N=462) per PSUM bank (+ a 1-row
runt); the kx tap shift is applied to the PSUM output AP (window 2-kx) so
every rhs offset stays even, and each drain is one strided op into the flat
64x64 layout.  Row-tiles are grouped 4/4/2 so each stationary weight load
feeds 4 matmuls.

Latency structure (vs the naive version):
 - input x is DMAd in 32-row chunks and s1 is signed per chunk, so conv1
   starts ~4us after kernel start instead of ~14us;
 - the epilogue fuses prelu into the pixel-unshuffle (Prelu activation with
   strided in / f32 out), split into top/bottom halves so half of it (and
   half of the output DMA) overlaps the remaining conv2 matmuls;
 - pad-ring memsets run on GpSimd, keeping DVE free for signs/adds.

Per-core pipeline (per image):
  DMA x chunk (fp16) -> s1 chunk = sign8(x + bias1_)      (DVE)
  conv1: 2 outgrps x 10 row-tiles x 9 taps DoubleRow matmuls -> PSUM
  t1    = psum * sc1 (ScalarE, fp16); xres = x + t1       (in-place, DVE)
  s2[h] = sign8(xres[h] + bias2_)   (emitted per group, overlaps group h+1)
  conv2: 10 row-tiles x 9 taps -> PSUM
  t2    = psum * sc2 + bias3 (ScalarE);  u = xres[:128] + t2  (DVE)
  epilogue (after rows 0-31 / 32-63 complete): 4 quadrant ops
  y_j   = prelu(u strided-quadrant view) -> f32 (ScalarE) -> DMA out
"""

import numpy as np
import ml_dtypes

import concourse.bass as bass
import concourse.mybir as mybir
from concourse import bacc
from concourse.tile import TileContext
from concourse.bass_utils import run_bass_kernel_spmd

B, C, H, W = 16, 256, 64, 64
NCORES = 8
BL = B // NCORES          # images per core
HW_ = H * W               # 4096
PW = W + 2                # 66 padded row width
HS = 4368                 # per-half stride in the sign buffer (16-aligned)
F32 = mybir.dt.float32
FP16 = mybir.dt.float16
FP8 = mybir.dt.float8e4
DR = mybir.MatmulPerfMode.DoubleRow

# row-tiles: 9 tiles of 7 output rows + 1 runt row
TILES = [(t * 7, 7) for t in range(9)] + [(63, 1)]

# engine selection knobs (see _prep_weights for the matching scale factors):
# 'v' = DVE tensor_scalar (is_ge - 0.5 -> +-0.5 signs, 2x folded in scales)
# 's' = ScalarE Sign activation (+-1 signs)
SIGN1_ENG = 'v'
SIGN2_ENG = 'v'

_CACHE = {}


def build_nc(reps=1, probe=None, use_b4=False):
    nc = bacc.Bacc()
    x_ext = nc.declare_dram_parameter("x", [BL, C, H, W], FP16, isOutput=False)
    w1_ext = nc.declare_dram_parameter("w1", [128, 18 * 256], FP8, isOutput=False)
    w2_ext = nc.declare_dram_parameter("w2", [128, 9 * 256], FP8, isOutput=False)
    coef_ext = nc.declare_dram_parameter("coef", [128, 10], F32, isOutput=False)
    out_ext = nc.declare_dram_parameter("out", [BL, 2 * C, H // 2, W // 2], F32,
                                        isOutput=True)

    Ident = mybir.ActivationFunctionType.Identity
    Alu = mybir.AluOpType

    with TileContext(nc) as tc:
        with (
            tc.tile_pool(name="weights", bufs=1) as pw,
            tc.tile_pool(name="xbuf", bufs=8) as px,
            tc.tile_pool(name="signs", bufs=6) as psn,
            tc.tile_pool(name="small", bufs=12) as pt,
            tc.tile_pool(name="ps", bufs=8, space="PSUM") as psum,
        ):
            coef_t = pw.tile([128, 10], F32, tag="coef")
            nc.sync.dma_start(out=coef_t[:, :], in_=coef_ext[:, :])
            w1_t = pw.tile([128, 18 * 256], FP8, tag="w1")
            w2_t = pw.tile([128, 9 * 256], FP8, tag="w2")

            st = [dict() for _ in range(BL)]

            ring_state = {"n": 0}

            def ring_zero(i, sv, h, eng=None):
                # pad ring of the 66x66 grid: top row (+1), bottom row, and
                # the left/right column pair of every row.  The ring stays
                # zero once written (signs only touch the interior), and the
                # pool has 6 slots, so only the first 6 tile allocations
                # need zeroing — later allocations land on already-zeroed
                # slots.  (Avoids false region deps stalling the signs.)
                if ring_state["n"] >= 12:       # 6 tiles x 2 halves
                    return
                ring_state["n"] += 1
                eng = eng or nc.gpsimd
                eng.memset(sv[:, h, 0:PW + 1], 0)
                eng.memset(sv[:, h, 65 * PW:HS], 0)
                lc = sv[:, h, 2 * PW - 1:2 * PW - 1 + 64 * PW].rearrange(
                    "p (r c) -> p r c", c=PW)
                eng.memset(lc[:, :, 0:2], 0)

            def sign_rows(i, sv, src, bias_col, h, r0, nr, eng):
                dst = sv[:, h, PW + r0 * PW:PW + (r0 + nr) * PW].rearrange(
                    "p (r c) -> p r c", c=PW)[:, :, 1:1 + W]
                s_src = src[:, r0 * W:(r0 + nr) * W].rearrange(
                    "p (y x) -> p y x", y=nr)
                if eng == 'v':
                    # s = (src >= -bias) - 0.5  -> {-0.5, +0.5} fp8
                    nc.vector.tensor_scalar(
                        dst, s_src, coef_t[:, bias_col + h:bias_col + h + 1],
                        0.5, op0=Alu.is_ge, op1=Alu.subtract)
                else:
                    # s = sign(src + bias) -> {-1, +1} fp8
                    nc.scalar.activation(
                        dst, s_src, mybir.ActivationFunctionType.Sign,
                        bias=coef_t[:, bias_col + h:bias_col + h + 1])

            def stage_A(i, first=False):
                # x DMA in row chunks; sign each chunk as it lands so conv1
                # can start early.  On the first image the weight DMAs are
                # interleaved after the chunks that gate the first matmuls
                # (w1 split per out-group) so x isn't stuck behind them.
                xs = []
                for h in range(2):
                    xb = px.tile([128, HW_], FP16, tag="x", name=f"x_{i}_{h}")
                    xs.append(xb)
                s = psn.tile([128, 2 * HS], FP8, tag="s", name=f"s_s1_{i}")
                sv = s[:, :].rearrange("p (h q) -> p h q", h=2, q=HS)
                st[i]["x"] = xs
                st[i]["s1"] = sv
                for h in range(2):
                    ring_zero(i, sv, h)
                chunks = [(0, 16), (16, 16), (32, 32)] if first else [(0, 32), (32, 32)]
                for chunk, (r0, nr) in enumerate(chunks):
                    for h in range(2):
                        nc.sync.dma_start(
                            out=xs[h][:, r0 * W:(r0 + nr) * W],
                            in_=x_ext[i, h * 128:(h + 1) * 128,
                                      r0:r0 + nr, :].rearrange(
                                          "c y x -> c (y x)"),
                        )
                    if first and chunk == 1:
                        nc.sync.dma_start(out=w1_t[:, :9 * 256],
                                          in_=w1_ext[:, :9 * 256])
                    for h in range(2):
                        sign_rows(i, sv, xs[h], 2, h, r0, nr, SIGN1_ENG)
                if first:
                    nc.sync.dma_start(out=w1_t[:, 9 * 256:],
                                      in_=w1_ext[:, 9 * 256:])
                    nc.sync.dma_start(out=w2_t[:, :], in_=w2_ext[:, :])

            def conv(i, sv, w_t, ngrp, drain, post_group=None):
                # tiles grouped 4/4/2 so each stationary weight feeds 4 MMs
                for g in range(ngrp):
                    for tb in (TILES[0:4], TILES[4:8], TILES[8:10]):
                        pts = []
                        for q, (y0, rows) in enumerate(tb):
                            pts.append(psum.tile([128, 512], F32, tag="ps",
                                                 name=f"ps_{i}_{g}_{y0}"))
                        for t in range(9):
                            if probe in ('nomm', 'justdma'):
                                break
                            ky, kx = t // 3, t % 3
                            wap = w_t[:, (g * 9 + t) * 256:(g * 9 + t + 1) * 256
                                      ].rearrange("p (h m) -> p h m", h=2)
                            for q, (y0, rows) in enumerate(tb):
                                n = rows * PW
                                off = PW * (y0 + ky)
                                nc.tensor.matmul(
                                    pts[q][:, 2 - kx:2 - kx + n], wap,
                                    sv[:, :, off:off + n],
                                    start=(t == 0), stop=(t == 8),
                                    perf_mode=DR,
                                )
                        for q, (y0, rows) in enumerate(tb):
                            drain(g, y0, rows, pts[q])
                    if post_group is not None:
                        post_group(g)

            def stage_B(i):  # conv1 + xres (in place into xb)
                xs = st[i]["x"]

                def drain(g, y0, rows, ps):
                    if probe in ('nodrain', 'nomm', 'justdma'):
                        return
                    n = rows * W
                    t1 = pt.tile([128, 448], FP16, tag="t1")
                    src = ps[:, 1:1 + rows * PW].rearrange(
                        "p (r c) -> p r c", c=PW)[:, :, 1:1 + W]
                    nc.scalar.mul(
                        t1[:, :n].rearrange("p (r c) -> p r c", c=W),
                        src, coef_t[:, g:g + 1])
                    xg = xs[g][:, y0 * W:y0 * W + n]
                    nc.vector.tensor_add(xg, xg, t1[:, :n])

                # s2 tile: each half is signed as soon as its group's
                # drains complete, overlapping the other group's matmuls
                s2 = psn.tile([128, 2 * HS], FP8, tag="s", name=f"s_s2_{i}")
                st[i]["s2"] = s2[:, :].rearrange("p (h q) -> p h q", h=2, q=HS)
                for h in range(2):
                    ring_zero(i, st[i]["s2"], h)

                def post_group(g):
                    sign_rows(i, st[i]["s2"], xs[g], 4, g, 0, 64, SIGN2_ENG)

                conv(i, st[i]["s1"], w1_t, 2, drain, post_group)

            def epilogue_part(i, x0, h0, nh):
                # fused prelu + pixel-unshuffle for output rows [h0, h0+nh)
                # of each of the 4 quadrants, + out DMA.
                uv = x0[:, :].rearrange("p (h2 r1 w2 r2) -> p r1 r2 h2 w2",
                                        h2=32, r1=2, w2=32, r2=2)
                od = out_ext[i, :, :, :].rearrange("(c j) y x -> c j y x", j=4)
                for j in range(4):
                    r1, r2 = j >> 1, j & 1
                    y = pt.tile([128, 512], F32, tag="y")
                    yv = y[:, :32 * nh].rearrange("p (a b) -> p a b",
                                                  a=nh, b=32)
                    nc.scalar.activation(
                        yv, uv[:, r1, r2, h0:h0 + nh, :],
                        mybir.ActivationFunctionType.Prelu,
                        alpha=coef_t[:, 8:9])
                    if use_b4:
                        nc.vector.tensor_scalar(
                            yv, yv, coef_t[:, 9:10], None, op0=Alu.add)
                    nc.sync.dma_start(
                        out=od[:, j, h0:h0 + nh, :], in_=yv)

            def stage_D(i):  # conv2 + epilogue + out DMA
                x0 = st[i]["x"][0]

                def drain(g, y0, rows, ps):
                    if probe in ('nodrain', 'noepi', 'nomm', 'justdma'):
                        return
                    n = rows * W
                    t2 = pt.tile([128, 448], FP16, tag="t1")
                    src = ps[:, 1:1 + rows * PW].rearrange(
                        "p (r c) -> p r c", c=PW)[:, :, 1:1 + W]
                    # t2 = psum * sc2 + bias3 on the DVE: ScalarE is the
                    # binding engine in the conv2/epilogue region.
                    nc.vector.tensor_scalar(
                        t2[:, :n].rearrange("p (r c) -> p r c", c=W),
                        src, coef_t[:, 6:7], coef_t[:, 7:8],
                        op0=Alu.mult, op1=Alu.add)
                    xb = x0[:, y0 * W:y0 * W + n]
                    nc.vector.tensor_add(xb, xb, t2[:, :n])   # u = t2 + xres
                    if y0 == 28:        # rows 0-34 done -> h2 [0,16)
                        epilogue_part(i, x0, 0, 16)
                    elif y0 == 42:      # rows 0-48 done -> h2 [16,24)
                        epilogue_part(i, x0, 16, 8)
                    elif y0 == 63:      # all rows done -> h2 [24,32)
                        epilogue_part(i, x0, 24, 8)

                conv(i, st[i]["s2"], w2_t, 1, drain)

            # software-pipelined emission: keep the PE busy across images
            for r in range(reps):
                stage_A(0, first=(r == 0))
                stage_A(1)
                stage_B(0)
                stage_B(1)
                stage_D(0)
                stage_D(1)

    nc.compile()
    return nc


def _prep_weights(inputs):
    w1 = np.asarray(inputs["conv1_w"], np.float32)          # [256,256,3,3]
    w2 = np.asarray(inputs["conv2_w"], np.float32)          # [128,256,3,3]
    # DVE signs are +-0.5 (not +-1), so those conv scales carry an extra 2x
    f1 = 2.0 if SIGN1_ENG == 'v' else 1.0
    f2 = 2.0 if SIGN2_ENG == 'v' else 1.0
    sc1 = (f1 * np.abs(w1).mean(axis=(1, 2, 3))
           * float(np.asarray(inputs["kw1"]))
           * float(np.asarray(inputs["ka1"]))).astype(np.float32)   # [256]
    sc2 = (f2 * np.abs(w2).mean(axis=(1, 2, 3))
           * float(np.asarray(inputs["kw2"]))
           * float(np.asarray(inputs["ka2"]))).astype(np.float32)   # [128]

    # w1b[i, g, t, h, o] = sign(w1)[g*128+o, h*128+i, t//3, t%3]
    sgn1 = np.sign(w1).reshape(2, 128, 2, 128, 9)           # [g,o,h,i,t]
    w1b = np.ascontiguousarray(sgn1.transpose(3, 0, 4, 2, 1)
                               ).reshape(128, 18 * 256).astype(
                                   ml_dtypes.float8_e4m3fn)
    sgn2 = np.sign(w2).reshape(128, 2, 128, 9)              # [o,h,i,t]
    w2b = np.ascontiguousarray(sgn2.transpose(2, 3, 1, 0)
                               ).reshape(128, 9 * 256).astype(
                                   ml_dtypes.float8_e4m3fn)

    coef = np.zeros((128, 10), np.float32)
    coef[:, 0] = sc1[:128]
    coef[:, 1] = sc1[128:]
    b1 = np.asarray(inputs["bias1_"], np.float32).reshape(C)
    b2 = np.asarray(inputs["bias2_"], np.float32).reshape(C)
    if SIGN1_ENG == 'v':
        coef[:, 2] = -b1[:128]        # is_ge threshold = -bias
        coef[:, 3] = -b1[128:]
    else:
        coef[:, 2] = b1[:128]         # Sign activation bias = +bias
        coef[:, 3] = b1[128:]
    if SIGN2_ENG == 'v':
        coef[:, 4] = -b2[:128]
        coef[:, 5] = -b2[128:]
    else:
        coef[:, 4] = b2[:128]
        coef[:, 5] = b2[128:]
    coef[:, 6] = sc2
    coef[:, 7] = np.asarray(inputs["bias3"], np.float32).reshape(C // 2)
    coef[:, 8] = np.asarray(inputs["prelu2_w"], np.float32)
    coef[:, 9] = np.asarray(inputs["bias4"], np.float32).reshape(C // 2)
    return w1b, w2b, coef


def kernel(**inputs):
    return kernel_with_results(**inputs)[0]


def kernel_with_results(trace=False, **inputs):
    x = np.ascontiguousarray(np.asarray(inputs["x"], np.float32).astype(np.float16))
    w1b, w2b, coef = _prep_weights(inputs)
    use_b4 = bool(np.any(np.asarray(inputs["bias4"])))

    key = ("nc", use_b4)
    if key not in _CACHE:
        _CACHE[key] = build_nc(use_b4=use_b4)
    nc = _CACHE[key]

    in_maps = [
        {"x": x[i * BL:(i + 1) * BL], "w1": w1b, "w2": w2b, "coef": coef}
        for i in range(NCORES)
    ]
    res = run_bass_kernel_spmd(nc, in_maps, core_ids=list(range(NCORES)),
                               trace=trace)
    out = np.concatenate([res.results[i]["out"] for i in range(NCORES)], axis=0)
    return out, res


# revision 17
# speedup vs baseline: 1.1793x; 1.1793x over previous
"""AdaBlock (binarized double-conv residual block) Trainium2 kernel.

Strategy
--------
Data-parallel over batch: 16 images across 8 NeuronCores (2 images/core), no
collectives.  The binarized convs are exact +-1 matmuls: a 3x3 conv is 9
shifted [Cin x spatial] matmuls accumulated in PSUM.  fp8 with
`perf_mode=DoubleRow` packs both 128-channel cin halves into one K=256
matmul (~90 ns / N=462 matmul measured on HW).  Sign activations are
computed as (x >= -b) - 0.5 on the VectorE (values +-0.5, exact in fp8; the
factor 2 is folded into the per-out-channel conv scales), intermediates are
fp16 to hit the DVE 2x/4x perf modes, PSUM accumulation stays fp32 so conv
sums are exact; overall mean rel err vs the fp32 reference ~2e-3.

Spatial layout: sign activations live in a zero-ring-padded 66x66 grid per
cin half (flat, half-stride 4368 so the DoubleRow rhs AP is [p, 2, N]).
Conv output is tiled over 7 padded rows (N=462) per PSUM bank (+ a 1-row
runt); the kx tap shift is applied to the PSUM output AP (window 2-kx) so
every rhs offset stays even, and each drain is one strided op into the flat
64x64 layout.  Row-tiles are grouped 4/4/2 so each stationary weight load
feeds 4 matmuls.

Latency structure (vs the naive version):
 - input x is DMAd in 32-row chunks and s1 is signed per chunk, so conv1
   starts ~4us after kernel start instead of ~14us;
 - the epilogue fuses prelu into the pixel-unshuffle (Prelu activation with
   strided in / f32 out), split into top/bottom halves so half of it (and
   half of the output DMA) overlaps the remaining conv2 matmuls;
 - pad-ring memsets run on GpSimd, keeping DVE free for signs/adds.

Per-core pipeline (per image):
  DMA x chunk (fp16) -> s1 chunk = sign8(x + bias1_)      (DVE)
  conv1: 2 outgrps x 10 row-tiles x 9 taps DoubleRow matmuls -> PSUM
  t1    = psum * sc1 (ScalarE, fp16); xres = x + t1       (in-place, DVE)
  s2[h] = sign8(xres[h] + bias2_)   (emitted per group, overlaps group h+1)
  conv2: 10 row-tiles x 9 taps -> PSUM
  t2    = psum * sc2 + bias3 (ScalarE);  u = xres[:128] + t2  (DVE)
  epilogue (after rows 0-31 / 32-63 complete): 4 quadrant ops
  y_j   = prelu(u strided-quadrant view) -> f32 (ScalarE) -> DMA out
"""

import numpy as np
import ml_dtypes

import concourse.bass as bass
import concourse.mybir as mybir
from concourse import bacc
from concourse.tile import TileContext
from concourse.bass_utils import run_bass_kernel_spmd

B, C, H, W = 16, 256, 64, 64
NCORES = 8
BL = B // NCORES          # images per core
HW_ = H * W               # 4096
PW = W + 2                # 66 padded row width
HS = 4368                 # per-half stride in the sign buffer (16-aligned)
F32 = mybir.dt.float32
FP16 = mybir.dt.float16
FP8 = mybir.dt.float8e4
DR = mybir.MatmulPerfMode.DoubleRow

# row-tiles: 9 tiles of 7 output rows + 1 runt row
TILES = [(t * 7, 7) for t in range(9)] + [(63, 1)]

# engine selection knobs (see _prep_weights for the matching scale factors):
# 'v' = DVE tensor_scalar (is_ge - 0.5 -> +-0.5 signs, 2x folded in scales)
# 's' = ScalarE Sign activation (+-1 signs)
SIGN1_ENG = 'v'
SIGN2_ENG = 'v'

_CACHE = {}


def build_nc(reps=1, probe=None, use_b4=False, t2_eng='v', ring_eng='dve'):
    nc = bacc.Bacc()
    x_ext = nc.declare_dram_parameter("x", [BL, C, H, W], FP16, isOutput=False)
    w1_ext = nc.declare_dram_parameter("w1", [128, 18 * 256], FP8, isOutput=False)
    w2_ext = nc.declare_dram_parameter("w2", [128, 9 * 256], FP8, isOutput=False)
    coef_ext = nc.declare_dram_parameter("coef", [128, 10], F32, isOutput=False)
    out_ext = nc.declare_dram_parameter("out", [BL, 2 * C, H // 2, W // 2], F32,
                                        isOutput=True)

    Ident = mybir.ActivationFunctionType.Identity
    Alu = mybir.AluOpType

    with TileContext(nc) as tc:
        with (
            tc.tile_pool(name="weights", bufs=1) as pw,
            tc.tile_pool(name="xbuf", bufs=8) as px,
            tc.tile_pool(name="signs", bufs=6) as psn,
            tc.tile_pool(name="small", bufs=12) as pt,
            tc.tile_pool(name="ps", bufs=8, space="PSUM") as psum,
        ):
            coef_t = pw.tile([128, 10], F32, tag="coef")
            nc.sync.dma_start(out=coef_t[:, :], in_=coef_ext[:, :])
            w1_t = pw.tile([128, 18 * 256], FP8, tag="w1")
            w2_t = pw.tile([128, 9 * 256], FP8, tag="w2")

            st = [dict() for _ in range(BL)]

            ring_state = {"n": 0}

            def ring_zero(i, sv, h, eng=None):
                # pad ring of the 66x66 grid: top row (+1), bottom row, and
                # the left/right column pair of every row.  The ring stays
                # zero once written (signs only touch the interior), and the
                # pool has 6 slots, so only the first 6 tile allocations
                # need zeroing — later allocations land on already-zeroed
                # slots.  (Avoids false region deps stalling the signs.)
                if ring_state["n"] >= 12:       # 6 tiles x 2 halves
                    return
                ring_state["n"] += 1
                eng = eng or (nc.gpsimd if ring_eng == 'pool' else nc.vector)
                eng.memset(sv[:, h, 0:PW + 1], 0)
                eng.memset(sv[:, h, 65 * PW:HS], 0)
                lc = sv[:, h, 2 * PW - 1:2 * PW - 1 + 64 * PW].rearrange(
                    "p (r c) -> p r c", c=PW)
                eng.memset(lc[:, :, 0:2], 0)

            def sign_rows(i, sv, src, bias_col, h, r0, nr, eng):
                dst = sv[:, h, PW + r0 * PW:PW + (r0 + nr) * PW].rearrange(
                    "p (r c) -> p r c", c=PW)[:, :, 1:1 + W]
                s_src = src[:, r0 * W:(r0 + nr) * W].rearrange(
                    "p (y x) -> p y x", y=nr)
                if eng == 'v':
                    # s = (src >= -bias) - 0.5  -> {-0.5, +0.5} fp8
                    nc.vector.tensor_scalar(
                        dst, s_src, coef_t[:, bias_col + h:bias_col + h + 1],
                        0.5, op0=Alu.is_ge, op1=Alu.subtract)
                else:
                    # s = sign(src + bias) -> {-1, +1} fp8
                    nc.scalar.activation(
                        dst, s_src, mybir.ActivationFunctionType.Sign,
                        bias=coef_t[:, bias_col + h:bias_col + h + 1])

            def stage_A(i, first=False):
                # x DMA in row chunks; sign each chunk as it lands so conv1
                # can start early.  On the first image the weight DMAs are
                # interleaved after the chunks that gate the first matmuls
                # (w1 split per out-group) so x isn't stuck behind them.
                xs = []
                for h in range(2):
                    xb = px.tile([128, HW_], FP16, tag="x", name=f"x_{i}_{h}")
                    xs.append(xb)
                s = psn.tile([128, 2 * HS], FP8, tag="s", name=f"s_s1_{i}")
                sv = s[:, :].rearrange("p (h q) -> p h q", h=2, q=HS)
                st[i]["x"] = xs
                st[i]["s1"] = sv
                for h in range(2):
                    ring_zero(i, sv, h)
                chunks = [(0, 16), (16, 16), (32, 32)] if first else [(0, 32), (32, 32)]
                for chunk, (r0, nr) in enumerate(chunks):
                    for h in range(2):
                        # on the first image, split the two cin halves
                        # across the SP HWDGE and the GpSimd SWDGE queues so
                        # they transfer in parallel (the Activation queue is
                        # blocked by the act-table load at kernel start)
                        dma_eng = nc.sync if h == 0 or not first else nc.gpsimd
                        dma_eng.dma_start(
                            out=xs[h][:, r0 * W:(r0 + nr) * W],
                            in_=x_ext[i, h * 128:(h + 1) * 128,
                                      r0:r0 + nr, :].rearrange(
                                          "c y x -> c (y x)"),
                        )
                    if first and chunk == 1:
                        # first matmuls only need w1's out-group 0
                        nc.gpsimd.dma_start(out=w1_t[:, :9 * 256],
                                            in_=w1_ext[:, :9 * 256])
                    for h in range(2):
                        sign_rows(i, sv, xs[h], 2, h, r0, nr, SIGN1_ENG)
                if first:
                    nc.gpsimd.dma_start(out=w1_t[:, 9 * 256:],
                                        in_=w1_ext[:, 9 * 256:])
                    nc.gpsimd.dma_start(out=w2_t[:, :], in_=w2_ext[:, :])

            def conv(i, sv, w_t, ngrp, drain, post_group=None):
                # tiles grouped 4/4/2 so each stationary weight feeds 4 MMs
                for g in range(ngrp):
                    for tb in (TILES[0:4], TILES[4:8], TILES[8:10]):
                        pts = []
                        for q, (y0, rows) in enumerate(tb):
                            pts.append(psum.tile([128, 512], F32, tag="ps",
                                                 name=f"ps_{i}_{g}_{y0}"))
                        for t in range(9):
                            if probe in ('nomm', 'justdma'):
                                break
                            ky, kx = t // 3, t % 3
                            wap = w_t[:, (g * 9 + t) * 256:(g * 9 + t + 1) * 256
                                      ].rearrange("p (h m) -> p h m", h=2)
                            for q, (y0, rows) in enumerate(tb):
                                n = rows * PW
                                off = PW * (y0 + ky)
                                nc.tensor.matmul(
                                    pts[q][:, 2 - kx:2 - kx + n], wap,
                                    sv[:, :, off:off + n],
                                    start=(t == 0), stop=(t == 8),
                                    perf_mode=DR,
                                )
                        for q, (y0, rows) in enumerate(tb):
                            drain(g, y0, rows, pts[q])
                    if post_group is not None:
                        post_group(g)

            def stage_B(i):  # conv1 + xres (in place into xb)
                xs = st[i]["x"]

                def drain(g, y0, rows, ps):
                    if probe in ('nodrain', 'nomm', 'justdma'):
                        return
                    n = rows * W
                    t1 = pt.tile([128, 448], FP16, tag="t1")
                    src = ps[:, 1:1 + rows * PW].rearrange(
                        "p (r c) -> p r c", c=PW)[:, :, 1:1 + W]
                    nc.scalar.mul(
                        t1[:, :n].rearrange("p (r c) -> p r c", c=W),
                        src, coef_t[:, g:g + 1])
                    xg = xs[g][:, y0 * W:y0 * W + n]
                    nc.vector.tensor_add(xg, xg, t1[:, :n])

                # s2 tile: each half is signed as soon as its group's
                # drains complete, overlapping the other group's matmuls
                s2 = psn.tile([128, 2 * HS], FP8, tag="s", name=f"s_s2_{i}")
                st[i]["s2"] = s2[:, :].rearrange("p (h q) -> p h q", h=2, q=HS)

                def post_group(g):
                    ring_zero(i, st[i]["s2"], g)
                    sign_rows(i, st[i]["s2"], xs[g], 4, g, 0, 64, SIGN2_ENG)

                conv(i, st[i]["s1"], w1_t, 2, drain, post_group)

            def epilogue_part(i, x0, h0, nh):
                # fused prelu + pixel-unshuffle for output rows [h0, h0+nh)
                # of each of the 4 quadrants, + out DMA.
                uv = x0[:, :].rearrange("p (h2 r1 w2 r2) -> p r1 r2 h2 w2",
                                        h2=32, r1=2, w2=32, r2=2)
                od = out_ext[i, :, :, :].rearrange("(c j) y x -> c j y x", j=4)
                for j in range(4):
                    r1, r2 = j >> 1, j & 1
                    y = pt.tile([128, 512], F32, tag="y")
                    yv = y[:, :32 * nh].rearrange("p (a b) -> p a b",
                                                  a=nh, b=32)
                    nc.scalar.activation(
                        yv, uv[:, r1, r2, h0:h0 + nh, :],
                        mybir.ActivationFunctionType.Prelu,
                        alpha=coef_t[:, 8:9])
                    if use_b4:
                        nc.vector.tensor_scalar(
                            yv, yv, coef_t[:, 9:10], None, op0=Alu.add)
                    nc.sync.dma_start(
                        out=od[:, j, h0:h0 + nh, :], in_=yv)

            def drain_D(i, y0, rows, ps):
                if probe in ('nodrain', 'noepi', 'nomm', 'justdma'):
                    return
                x0 = st[i]["x"][0]
                n = rows * W
                t2 = pt.tile([128, 448], FP16, tag="t1")
                src = ps[:, 1:1 + rows * PW].rearrange(
                    "p (r c) -> p r c", c=PW)[:, :, 1:1 + W]
                # t2 = psum * sc2 + bias3; engine selectable (ScalarE is
                # loaded in the conv2/epilogue region, but DVE PSUM reads
                # may be slower on HW).  The last tiles go to ScalarE
                # regardless: at the tail the DVE is the serial drain->add
                # chain, while ScalarE idles between epilogue batches.
                if t2_eng == 'v' and y0 < 49:
                    nc.vector.tensor_scalar(
                        t2[:, :n].rearrange("p (r c) -> p r c", c=W),
                        src, coef_t[:, 6:7], coef_t[:, 7:8],
                        op0=Alu.mult, op1=Alu.add)
                else:
                    nc.scalar.activation(
                        t2[:, :n].rearrange("p (r c) -> p r c", c=W),
                        src, Ident, bias=coef_t[:, 7:8],
                        scale=coef_t[:, 6:7])
                xb = x0[:, y0 * W:y0 * W + n]
                nc.vector.tensor_add(xb, xb, t2[:, :n])   # u = t2 + xres
                if y0 == 21:        # rows 0-27 done -> h2 [0,13)
                    epilogue_part(i, x0, 0, 13)
                elif y0 == 49:      # rows 0-55 done -> h2 [13,27)
                    epilogue_part(i, x0, 13, 14)
                elif y0 == 63:      # all rows done -> h2 [27,32)
                    epilogue_part(i, x0, 27, 5)

            def stage_D_pair(i0, i1):
                # conv2 for both images, interleaved at tile-group level:
                # each image's drains/epilogue then overlap the OTHER
                # image's matmuls, so the tail is only the last epilogue
                # batch instead of a whole image's epilogue.
                for tb in (TILES[0:4], TILES[4:8], TILES[8:10]):
                    for i in (i0, i1):
                        sv = st[i]["s2"]
                        pts = []
                        for q, (y0, rows) in enumerate(tb):
                            pts.append(psum.tile([128, 512], F32, tag="ps",
                                                 name=f"ps2_{i}_{y0}"))
                        for t in range(9):
                            if probe in ('nomm', 'justdma'):
                                break
                            ky, kx = t // 3, t % 3
                            wap = w2_t[:, t * 256:(t + 1) * 256
                                       ].rearrange("p (h m) -> p h m", h=2)
                            for q, (y0, rows) in enumerate(tb):
                                n = rows * PW
                                off = PW * (y0 + ky)
                                nc.tensor.matmul(
                                    pts[q][:, 2 - kx:2 - kx + n], wap,
                                    sv[:, :, off:off + n],
                                    start=(t == 0), stop=(t == 8),
                                    perf_mode=DR,
                                )
                        for q, (y0, rows) in enumerate(tb):
                            drain_D(i, y0, rows, pts[q])

            # software-pipelined emission: keep the PE busy across images
            for r in range(reps):
                stage_A(0, first=(r == 0))
                stage_A(1)
                stage_B(0)
                stage_B(1)
                stage_D_pair(0, 1)

    nc.compile()
    return nc


def _prep_weights(inputs):
    w1 = np.asarray(inputs["conv1_w"], np.float32)          # [256,256,3,3]
    w2 = np.asarray(inputs["conv2_w"], np.float32)          # [128,256,3,3]
    # DVE signs are +-0.5 (not +-1), so those conv scales carry an extra 2x
    f1 = 2.0 if SIGN1_ENG == 'v' else 1.0
    f2 = 2.0 if SIGN2_ENG == 'v' else 1.0
    sc1 = (f1 * np.abs(w1).mean(axis=(1, 2, 3))
           * float(np.asarray(inputs["kw1"]))
           * float(np.asarray(inputs["ka1"]))).astype(np.float32)   # [256]
    sc2 = (f2 * np.abs(w2).mean(axis=(1, 2, 3))
           * float(np.asarray(inputs["kw2"]))
           * float(np.asarray(inputs["ka2"]))).astype(np.float32)   # [128]

    # w1b[i, g, t, h, o] = sign(w1)[g*128+o, h*128+i, t//3, t%3]
    sgn1 = np.sign(w1).reshape(2, 128, 2, 128, 9)           # [g,o,h,i,t]
    w1b = np.ascontiguousarray(sgn1.transpose(3, 0, 4, 2, 1)
                               ).reshape(128, 18 * 256).astype(
                                   ml_dtypes.float8_e4m3fn)
    sgn2 = np.sign(w2).reshape(128, 2, 128, 9)              # [o,h,i,t]
    w2b = np.ascontiguousarray(sgn2.transpose(2, 3, 1, 0)
                               ).reshape(128, 9 * 256).astype(
                                   ml_dtypes.float8_e4m3fn)

    coef = np.zeros((128, 10), np.float32)
    coef[:, 0] = sc1[:128]
    coef[:, 1] = sc1[128:]
    b1 = np.asarray(inputs["bias1_"], np.float32).reshape(C)
    b2 = np.asarray(inputs["bias2_"], np.float32).reshape(C)
    if SIGN1_ENG == 'v':
        coef[:, 2] = -b1[:128]        # is_ge threshold = -bias
        coef[:, 3] = -b1[128:]
    else:
        coef[:, 2] = b1[:128]         # Sign activation bias = +bias
        coef[:, 3] = b1[128:]
    if SIGN2_ENG == 'v':
        coef[:, 4] = -b2[:128]
        coef[:, 5] = -b2[128:]
    else:
        coef[:, 4] = b2[:128]
        coef[:, 5] = b2[128:]
    coef[:, 6] = sc2
    coef[:, 7] = np.asarray(inputs["bias3"], np.float32).reshape(C // 2)
    coef[:, 8] = np.asarray(inputs["prelu2_w"], np.float32)
    coef[:, 9] = np.asarray(inputs["bias4"], np.float32).reshape(C // 2)
    return w1b, w2b, coef


def kernel(**inputs):
    return kernel_with_results(**inputs)[0]


def kernel_with_results(trace=False, **inputs):
    x = np.ascontiguousarray(np.asarray(inputs["x"], np.float32).astype(np.float16))
    w1b, w2b, coef = _prep_weights(inputs)
    use_b4 = bool(np.any(np.asarray(inputs["bias4"])))

    key = ("nc", use_b4)
    if key not in _CACHE:
        _CACHE[key] = build_nc(use_b4=use_b4)
    nc = _CACHE[key]

    in_maps = [
        {"x": x[i * BL:(i + 1) * BL], "w1": w1b, "w2": w2b, "coef": coef}
        for i in range(NCORES)
    ]
    res = run_bass_kernel_spmd(nc, in_maps, core_ids=list(range(NCORES)),
                               trace=trace)
    out = np.concatenate([res.results[i]["out"] for i in range(NCORES)], axis=0)
    return out, res


# revision 26
# speedup vs baseline: 1.3036x; 1.1053x over previous
"""AdaBlock (binarized double-conv residual block) Trainium2 kernel.

Strategy
--------
Data-parallel over batch: 16 images across 8 NeuronCores (2 images/core), no
collectives.  The binarized convs are exact +-1 matmuls: a 3x3 conv is 9
shifted [Cin x spatial] matmuls accumulated in PSUM.  fp8 with
`perf_mode=DoubleRow` packs both 128-channel cin halves into one K=256
matmul (~90 ns / N=462 matmul measured on HW).  Sign activations are
computed as (x >= -b) - 0.5 on the VectorE (values +-0.5, exact in fp8; the
factor 2 is folded into the per-out-channel conv scales), intermediates are
fp16 to hit the DVE 2x/4x perf modes, PSUM accumulation stays fp32 so conv
sums are exact; overall mean rel err vs the fp32 reference ~2e-3.

Spatial layout: sign activations live in a zero-ring-padded 66x66 grid per
cin half (flat, half-stride 4368 so the DoubleRow rhs AP is [p, 2, N]).
Conv output is tiled over 7 padded rows (N=462) per PSUM bank (+ a 1-row
runt); the kx tap shift is applied to the PSUM output AP (window 2-kx) so
every rhs offset stays even, and each drain is one strided op into the flat
64x64 layout.  Row-tiles are grouped 4/4/2 so each stationary weight load
feeds 4 matmuls.

Latency structure (vs the naive version):
 - input x is DMAd in 32-row chunks and s1 is signed per chunk, so conv1
   starts ~4us after kernel start instead of ~14us;
 - the epilogue fuses prelu into the pixel-unshuffle (Prelu activation with
   strided in / f32 out), split into top/bottom halves so half of it (and
   half of the output DMA) overlaps the remaining conv2 matmuls;
 - pad-ring memsets run on GpSimd, keeping DVE free for signs/adds.

Per-core pipeline (per image):
  DMA x chunk (fp16) -> s1 chunk = sign8(x + bias1_)      (DVE)
  conv1: 2 outgrps x 10 row-tiles x 9 taps DoubleRow matmuls -> PSUM
  t1    = psum * sc1 (ScalarE, fp16); xres = x + t1       (in-place, DVE)
  s2[h] = sign8(xres[h] + bias2_)   (emitted per group, overlaps group h+1)
  conv2: 10 row-tiles x 9 taps -> PSUM
  t2    = psum * sc2 + bias3 (ScalarE);  u = xres[:128] + t2  (DVE)
  epilogue (after rows 0-31 / 32-63 complete): 4 quadrant ops
  y_j   = prelu(u strided-quadrant view) -> f32 (ScalarE) -> DMA out
"""

import numpy as np
import ml_dtypes

import concourse.bass as bass
import concourse.mybir as mybir
from concourse import bacc
from concourse.tile import TileContext
from concourse.bass_utils import run_bass_kernel_spmd

B, C, H, W = 16, 256, 64, 64
NCORES = 8
BL = B // NCORES          # images per core
HW_ = H * W               # 4096
PW = W + 2                # 66 padded row width
HS = 4368                 # per-half stride in the sign buffer (16-aligned)
F32 = mybir.dt.float32
FP16 = mybir.dt.float16
FP8 = mybir.dt.float8e4
DR = mybir.MatmulPerfMode.DoubleRow

# row-tiles: 9 tiles of 7 output rows + 1 runt row
TILES = [(t * 7, 7) for t in range(9)] + [(63, 1)]

# engine selection knobs (see _prep_weights for the matching scale factors):
# 'v' = DVE tensor_scalar (is_ge - 0.5 -> +-0.5 signs, 2x folded in scales)
# 's' = ScalarE Sign activation (+-1 signs)
SIGN1_ENG = 'v'
SIGN2_ENG = 'v'

_CACHE = {}


def build_nc(reps=1, probe=None, use_b4=False, t2_eng='v', ring_eng='dve'):
    nc = bacc.Bacc()
    x_ext = nc.declare_dram_parameter("x", [BL, C, H, W], FP16, isOutput=False)
    w1_ext = nc.declare_dram_parameter("w1", [128, 18 * 256], FP8, isOutput=False)
    w2_ext = nc.declare_dram_parameter("w2", [128, 9 * 256], FP8, isOutput=False)
    coef_ext = nc.declare_dram_parameter("coef", [128, 10], F32, isOutput=False)
    out_ext = nc.declare_dram_parameter("out", [BL, 2 * C, H // 2, W // 2], F32,
                                        isOutput=True)

    Ident = mybir.ActivationFunctionType.Identity
    Alu = mybir.AluOpType

    with TileContext(nc) as tc:
        with (
            tc.tile_pool(name="weights", bufs=1) as pw,
            tc.tile_pool(name="xbuf", bufs=6) as px,
            tc.tile_pool(name="signs", bufs=6) as psn,
            tc.tile_pool(name="small", bufs=12) as pt,
            tc.tile_pool(name="ytile", bufs=4) as py,
            tc.tile_pool(name="ps", bufs=8, space="PSUM") as psum,
        ):
            coef_t = pw.tile([128, 10], F32, tag="coef")
            nc.sync.dma_start(out=coef_t[:, :], in_=coef_ext[:, :])
            w1_t = pw.tile([128, 18 * 256], FP8, tag="w1")
            w2_t = pw.tile([128, 9 * 256], FP8, tag="w2")

            st = [dict() for _ in range(BL)]

            ring_state = {"n": 0}

            def ring_zero(i, sv, h, eng=None):
                # pad ring of the 66x66 grid: top row (+1), bottom row, and
                # the left/right column pair of every row.  The ring stays
                # zero once written (signs only touch the interior), and the
                # pool has 6 slots, so only the first 6 tile allocations
                # need zeroing — later allocations land on already-zeroed
                # slots.  (Avoids false region deps stalling the signs.)
                if ring_state["n"] >= 12:       # 6 tiles x 2 halves
                    return
                ring_state["n"] += 1
                eng = eng or (nc.gpsimd if ring_eng == 'pool' else nc.vector)
                eng.memset(sv[:, h, 0:PW + 1], 0)
                eng.memset(sv[:, h, 65 * PW:HS], 0)
                lc = sv[:, h, 2 * PW - 1:2 * PW - 1 + 64 * PW].rearrange(
                    "p (r c) -> p r c", c=PW)
                eng.memset(lc[:, :, 0:2], 0)

            def sign_rows(i, sv, src, bias_col, h, r0, nr, eng):
                dst = sv[:, h, PW + r0 * PW:PW + (r0 + nr) * PW].rearrange(
                    "p (r c) -> p r c", c=PW)[:, :, 1:1 + W]
                s_src = src[:, r0 * W:(r0 + nr) * W].rearrange(
                    "p (y x) -> p y x", y=nr)
                if eng == 'v':
                    # s = (src >= -bias) - 0.5  -> {-0.5, +0.5} fp8
                    nc.vector.tensor_scalar(
                        dst, s_src, coef_t[:, bias_col + h:bias_col + h + 1],
                        0.5, op0=Alu.is_ge, op1=Alu.subtract)
                else:
                    # s = sign(src + bias) -> {-1, +1} fp8
                    nc.scalar.activation(
                        dst, s_src, mybir.ActivationFunctionType.Sign,
                        bias=coef_t[:, bias_col + h:bias_col + h + 1])

            def stage_A(i, first=False):
                # x DMA in row chunks; sign each chunk as it lands so conv1
                # can start early.  On the first image the weight DMAs are
                # interleaved after the chunks that gate the first matmuls
                # (w1 split per out-group) so x isn't stuck behind them.
                xs = []
                for h in range(2):
                    xb = px.tile([128, HW_], FP16, tag="x", name=f"x_{i}_{h}")
                    xs.append(xb)
                s = psn.tile([128, 2 * HS], FP8, tag="s", name=f"s_s1_{i}")
                sv = s[:, :].rearrange("p (h q) -> p h q", h=2, q=HS)
                st[i]["x"] = xs
                st[i]["s1"] = sv
                for h in range(2):
                    ring_zero(i, sv, h)
                chunks = [(0, 16), (16, 16), (32, 32)] if first else [(0, 32), (32, 32)]
                for chunk, (r0, nr) in enumerate(chunks):
                    for h in range(2):
                        # x chunks on the SP HWDGE queue; weights go via the
                        # Activation queue so they don't serialize behind x
                        # (gpsimd SWDGE is pathologically slow on HW)
                        dma_eng = nc.sync
                        dma_eng.dma_start(
                            out=xs[h][:, r0 * W:(r0 + nr) * W],
                            in_=x_ext[i, h * 128:(h + 1) * 128,
                                      r0:r0 + nr, :].rearrange(
                                          "c y x -> c (y x)"),
                        )
                    if first and chunk == 1:
                        # first matmuls only need w1's out-group 0
                        nc.scalar.dma_start(out=w1_t[:, :9 * 256],
                                            in_=w1_ext[:, :9 * 256])
                    for h in range(2):
                        sign_rows(i, sv, xs[h], 2, h, r0, nr, SIGN1_ENG)
                if first:
                    nc.scalar.dma_start(out=w1_t[:, 9 * 256:],
                                        in_=w1_ext[:, 9 * 256:])
                    nc.scalar.dma_start(out=w2_t[:, :], in_=w2_ext[:, :])

            def drain_B(i, g, y0, rows, ps):
                if probe in ('nodrain', 'nomm', 'justdma'):
                    return
                xs = st[i]["x"]
                n = rows * W
                t1 = pt.tile([128, 448], FP16, tag="t1")
                src = ps[:, 1:1 + rows * PW].rearrange(
                    "p (r c) -> p r c", c=PW)[:, :, 1:1 + W]
                nc.scalar.mul(
                    t1[:, :n].rearrange("p (r c) -> p r c", c=W),
                    src, coef_t[:, g:g + 1])
                xg = xs[g][:, y0 * W:y0 * W + n]
                nc.vector.tensor_add(xg, xg, t1[:, :n])
                # s2 half g is signed in 32-row chunks as xres completes,
                # so conv2 can start as early as possible
                if y0 == 28:
                    ring_zero(i, st[i]["s2"], g)
                    sign_rows(i, st[i]["s2"], xs[g], 4, g, 0, 32, SIGN2_ENG)
                elif y0 == 63:
                    sign_rows(i, st[i]["s2"], xs[g], 4, g, 32, 32, SIGN2_ENG)

            def prep_B(i):
                s2 = psn.tile([128, 2 * HS], FP8, tag="s", name=f"s_s2_{i}")
                st[i]["s2"] = s2[:, :].rearrange("p (h q) -> p h q", h=2, q=HS)

            def epilogue_part(i, x0, h0, nh):
                # fused prelu + pixel-unshuffle for output rows [h0, h0+nh)
                # of each of the 4 quadrants; one shared out DMA per chunk
                # (4 strided-in Prelu ops into one j-major tile).
                uv = x0[:, :].rearrange("p (h2 r1 w2 r2) -> p r1 r2 h2 w2",
                                        h2=32, r1=2, w2=32, r2=2)
                od = out_ext[i, :, :, :].rearrange("(c j) y x -> c j y x", j=4)
                y = py.tile([128, 2048], F32, tag="y")
                yv = y[:, :4 * nh * 32].rearrange("p (j a b) -> p j a b",
                                                  j=4, a=nh, b=32)
                for j in range(4):
                    r1, r2 = j >> 1, j & 1
                    nc.scalar.activation(
                        yv[:, j, :, :], uv[:, r1, r2, h0:h0 + nh, :],
                        mybir.ActivationFunctionType.Prelu,
                        alpha=coef_t[:, 8:9])
                    if use_b4:
                        nc.vector.tensor_scalar(
                            yv[:, j, :, :], yv[:, j, :, :],
                            coef_t[:, 9:10], None, op0=Alu.add)
                nc.sync.dma_start(out=od[:, :, h0:h0 + nh, :], in_=yv)

            def drain_D(i, y0, rows, ps):
                if probe in ('nodrain', 'noepi', 'nomm', 'justdma'):
                    return
                x0 = st[i]["x"][0]
                n = rows * W
                t2 = pt.tile([128, 448], FP16, tag="t1")
                src = ps[:, 1:1 + rows * PW].rearrange(
                    "p (r c) -> p r c", c=PW)[:, :, 1:1 + W]
                # t2 = psum * sc2 + bias3; engine selectable (ScalarE is
                # loaded in the conv2/epilogue region, but DVE PSUM reads
                # may be slower on HW).  The last tiles go to ScalarE
                # regardless: at the tail the DVE is the serial drain->add
                # chain, while ScalarE idles between epilogue batches.
                if t2_eng == 'v' and y0 < 49:
                    nc.vector.tensor_scalar(
                        t2[:, :n].rearrange("p (r c) -> p r c", c=W),
                        src, coef_t[:, 6:7], coef_t[:, 7:8],
                        op0=Alu.mult, op1=Alu.add)
                else:
                    nc.scalar.activation(
                        t2[:, :n].rearrange("p (r c) -> p r c", c=W),
                        src, Ident, bias=coef_t[:, 7:8],
                        scale=coef_t[:, 6:7])
                xb = x0[:, y0 * W:y0 * W + n]
                nc.vector.tensor_add(xb, xb, t2[:, :n])   # u = t2 + xres
                # epilogue in 3 chunks as rows complete (h2 <= (y0+5)/2)
                epi = {21: (0, 13), 49: (13, 14), 63: (27, 5)}.get(y0)
                if epi is not None:
                    epilogue_part(i, x0, epi[0], epi[1])

            def conv_unit(i, kind, g, tbi):
                # one PE work unit: 9-tap DoubleRow matmuls for one 4/4/2
                # row-tile group of conv<kind> for image i (out-group g),
                # followed by the tile drains.
                sv = st[i]["s1"] if kind == 1 else st[i]["s2"]
                w_t = w1_t if kind == 1 else w2_t
                tb = (TILES[0:4], TILES[4:8], TILES[8:10])[tbi]
                pts = []
                for q, (y0, rows) in enumerate(tb):
                    pts.append(psum.tile([128, 512], F32, tag="ps",
                                         name=f"ps{kind}_{i}_{g}_{y0}"))
                for t in range(9):
                    if probe in ('nomm', 'justdma'):
                        break
                    ky, kx = t // 3, t % 3
                    col0 = (g * 9 + t) * 256 if kind == 1 else t * 256
                    wap = w_t[:, col0:col0 + 256].rearrange(
                        "p (h m) -> p h m", h=2)
                    for q, (y0, rows) in enumerate(tb):
                        n = rows * PW
                        off = PW * (y0 + ky)
                        nc.tensor.matmul(
                            pts[q][:, 2 - kx:2 - kx + n], wap,
                            sv[:, :, off:off + n],
                            start=(t == 0), stop=(t == 8),
                            perf_mode=DR,
                        )
                for q, (y0, rows) in enumerate(tb):
                    if kind == 1:
                        drain_B(i, g, y0, rows, pts[q])
                    else:
                        drain_D(i, y0, rows, pts[q])

            # Software-pipelined emission.  conv2(0) units are interleaved
            # into the conv1(1) window so image 0's drains/epilogue spread
            # over ~26us of matmuls instead of 7.4; conv2(1) follows with
            # only its last tile-group's epilogue as the tail.
            for r in range(reps):
                stage_A(0, first=(r == 0))
                stage_A(1)
                prep_B(0)
                for g in (0, 1):
                    for tbi in (0, 1, 2):
                        conv_unit(0, 1, g, tbi)
                prep_B(1)
                for (i, kind, g, tbi) in [
                    (1, 1, 0, 0), (1, 1, 0, 1), (0, 2, 0, 0),
                    (1, 1, 0, 2), (1, 1, 1, 0), (0, 2, 0, 1),
                    (1, 1, 1, 1), (1, 1, 1, 2), (0, 2, 0, 2),
                ]:
                    conv_unit(i, kind, g, tbi)
                for tbi in (0, 1, 2):
                    conv_unit(1, 2, 0, tbi)

    nc.compile()
    return nc


def _prep_weights(inputs):
    w1 = np.asarray(inputs["conv1_w"], np.float32)          # [256,256,3,3]
    w2 = np.asarray(inputs["conv2_w"], np.float32)          # [128,256,3,3]
    # DVE signs are +-0.5 (not +-1), so those conv scales carry an extra 2x
    f1 = 2.0 if SIGN1_ENG == 'v' else 1.0
    f2 = 2.0 if SIGN2_ENG == 'v' else 1.0
    sc1 = (f1 * np.abs(w1).mean(axis=(1, 2, 3))
           * float(np.asarray(inputs["kw1"]))
           * float(np.asarray(inputs["ka1"]))).astype(np.float32)   # [256]
    sc2 = (f2 * np.abs(w2).mean(axis=(1, 2, 3))
           * float(np.asarray(inputs["kw2"]))
           * float(np.asarray(inputs["ka2"]))).astype(np.float32)   # [128]

    # w1b[i, g, t, h, o] = sign(w1)[g*128+o, h*128+i, t//3, t%3]
    sgn1 = np.sign(w1).reshape(2, 128, 2, 128, 9)           # [g,o,h,i,t]
    w1b = np.ascontiguousarray(sgn1.transpose(3, 0, 4, 2, 1)
                               ).reshape(128, 18 * 256).astype(
                                   ml_dtypes.float8_e4m3fn)
    sgn2 = np.sign(w2).reshape(128, 2, 128, 9)              # [o,h,i,t]
    w2b = np.ascontiguousarray(sgn2.transpose(2, 3, 1, 0)
                               ).reshape(128, 9 * 256).astype(
                                   ml_dtypes.float8_e4m3fn)

    coef = np.zeros((128, 10), np.float32)
    coef[:, 0] = sc1[:128]
    coef[:, 1] = sc1[128:]
    b1 = np.asarray(inputs["bias1_"], np.float32).reshape(C)
    b2 = np.asarray(inputs["bias2_"], np.float32).reshape(C)
    if SIGN1_ENG == 'v':
        coef[:, 2] = -b1[:128]        # is_ge threshold = -bias
        coef[:, 3] = -b1[128:]
    else:
        coef[:, 2] = b1[:128]         # Sign activation bias = +bias
        coef[:, 3] = b1[128:]
    if SIGN2_ENG == 'v':
        coef[:, 4] = -b2[:128]
        coef[:, 5] = -b2[128:]
    else:
        coef[:, 4] = b2[:128]
        coef[:, 5] = b2[128:]
    coef[:, 6] = sc2
    coef[:, 7] = np.asarray(inputs["bias3"], np.float32).reshape(C // 2)
    coef[:, 8] = np.asarray(inputs["prelu2_w"], np.float32)
    coef[:, 9] = np.asarray(inputs["bias4"], np.float32).reshape(C // 2)
    return w1b, w2b, coef


def kernel(**inputs):
    return kernel_with_results(**inputs)[0]


def kernel_with_results(trace=False, **inputs):
    x = np.ascontiguousarray(np.asarray(inputs["x"], np.float32).astype(np.float16))
    w1b, w2b, coef = _prep_weights(inputs)
    use_b4 = bool(np.any(np.asarray(inputs["bias4"])))

    key = ("nc", use_b4)
    if key not in _CACHE:
        _CACHE[key] = build_nc(use_b4=use_b4)
    nc = _CACHE[key]

    in_maps = [
        {"x": x[i * BL:(i + 1) * BL], "w1": w1b, "w2": w2b, "coef": coef}
        for i in range(NCORES)
    ]
    res = run_bass_kernel_spmd(nc, in_maps, core_ids=list(range(NCORES)),
                               trace=trace)
    out = np.concatenate([res.results[i]["out"] for i in range(NCORES)], axis=0)
    return out, res


# revision 33
# speedup vs baseline: 1.3356x; 1.0246x over previous
"""AdaBlock (binarized double-conv residual block) Trainium2 kernel.

Strategy
--------
Data-parallel over batch: 16 images across 8 NeuronCores (2 images/core), no
collectives.  The binarized convs are exact +-1 matmuls: a 3x3 conv is 9
shifted [Cin x spatial] matmuls accumulated in PSUM.  fp8 with
`perf_mode=DoubleRow` packs both 128-channel cin halves into one K=256
matmul (~90 ns / N=462 matmul measured on HW).  Sign activations are
computed as (x >= -b) - 0.5 on the VectorE (values +-0.5, exact in fp8; the
factor 2 is folded into the per-out-channel conv scales), intermediates are
fp16 to hit the DVE 2x/4x perf modes, PSUM accumulation stays fp32 so conv
sums are exact; overall mean rel err vs the fp32 reference ~2e-3.

Spatial layout: sign activations live in a zero-ring-padded 66x66 grid per
cin half (flat, half-stride 4368 so the DoubleRow rhs AP is [p, 2, N]).
Conv output is tiled over 7 padded rows (N=462) per PSUM bank (+ a 1-row
runt); the kx tap shift is applied to the PSUM output AP (window 2-kx) so
every rhs offset stays even, and each drain is one strided op into the flat
64x64 layout.  Row-tiles are grouped 4/4/2 so each stationary weight load
feeds 4 matmuls.

Latency structure (HW-microbenchmarked: elemwise ops are fast on HW — DVE
sign 429ns, strided Prelu 54ns — so the kernel is PE-bound plus dependency
latency; the scheduling below minimizes startup, stalls, and tail):
 - input x is DMAd in 16/16/32-row chunks (s1 signed per chunk) with the
   weight DMAs on the Activation DGE queue, so conv1 starts ~4us in;
 - conv2(0) tile-groups are interleaved into the conv1(1) window, so image
   0's drains/epilogue spread over ~26us of matmuls; conv2(1) follows with
   only its last tile-group's epilogue as the tail;
 - s2 halves are signed in 32-row chunks straight from the conv1 drain
   callbacks (y0==28/63);
 - the epilogue fuses prelu into the pixel-unshuffle (strided-in Prelu,
   f32 out) in 3 row-chunks per image, each with ONE merged 4-quadrant
   output DMA;
 - GpSimd is avoided entirely (software ops are ~100x slower than modeled);
   pad-ring memsets run on DVE and are skipped once all 6 sign-pool slots
   have been zeroed (rings stay zero across reps).

Per-core pipeline (per image):
  DMA x chunk (fp16) -> s1 chunk = sign8(x + bias1_)      (DVE)
  conv1: 2 outgrps x 10 row-tiles x 9 taps DoubleRow matmuls -> PSUM
  t1    = psum * sc1 (ScalarE, fp16); xres = x + t1       (in-place, DVE)
  s2[g] = sign8(xres[g] + bias2_)   (32-row chunks, from drain callbacks)
  conv2: 10 row-tiles x 9 taps -> PSUM (interleaved across images)
  t2    = psum * sc2 + bias3 (DVE; last tiles ScalarE); u = xres[:128] + t2
  epilogue chunks (h2 0:13 / 13:27 / 27:32): 4 strided Prelu ops -> one
  j-major f32 tile -> single merged DMA out
"""

import numpy as np
import ml_dtypes

import concourse.bass as bass
import concourse.mybir as mybir
from concourse import bacc
from concourse.tile import TileContext
from concourse.bass_utils import run_bass_kernel_spmd

B, C, H, W = 16, 256, 64, 64
NCORES = 8
BL = B // NCORES          # images per core
HW_ = H * W               # 4096
PW = W + 2                # 66 padded row width
HS = 4368                 # per-half stride in the sign buffer (16-aligned)
F32 = mybir.dt.float32
FP16 = mybir.dt.float16
FP8 = mybir.dt.float8e4
DR = mybir.MatmulPerfMode.DoubleRow

# row-tiles: 9 tiles of 7 output rows + 1 runt row
TILES = [(t * 7, 7) for t in range(9)] + [(63, 1)]

# engine selection knobs (see _prep_weights for the matching scale factors):
# 'v' = DVE tensor_scalar (is_ge - 0.5 -> +-0.5 signs, 2x folded in scales)
# 's' = ScalarE Sign activation (+-1 signs)
SIGN1_ENG = 'v'
SIGN2_ENG = 'v'

_CACHE = {}


def build_nc(reps=1, probe=None, use_b4=False, t2_eng='v', ring_eng='dve'):
    nc = bacc.Bacc()
    x_ext = nc.declare_dram_parameter("x", [BL, C, H, W], FP16, isOutput=False)
    w1_ext = nc.declare_dram_parameter("w1", [128, 18 * 256], FP8, isOutput=False)
    w2_ext = nc.declare_dram_parameter("w2", [128, 9 * 256], FP8, isOutput=False)
    coef_ext = nc.declare_dram_parameter("coef", [128, 10], F32, isOutput=False)
    out_ext = nc.declare_dram_parameter("out", [BL, 2 * C, H // 2, W // 2], F32,
                                        isOutput=True)

    Ident = mybir.ActivationFunctionType.Identity
    Alu = mybir.AluOpType

    with TileContext(nc) as tc:
        with (
            tc.tile_pool(name="weights", bufs=1) as pw,
            tc.tile_pool(name="xbuf", bufs=6) as px,
            tc.tile_pool(name="signs", bufs=6) as psn,
            tc.tile_pool(name="small", bufs=12) as pt,
            tc.tile_pool(name="ytile", bufs=4) as py,
            tc.tile_pool(name="ps", bufs=8, space="PSUM") as psum,
        ):
            coef_t = pw.tile([128, 10], F32, tag="coef")
            nc.sync.dma_start(out=coef_t[:, :], in_=coef_ext[:, :])
            w1_t = pw.tile([128, 18 * 256], FP8, tag="w1")
            w2_t = pw.tile([128, 9 * 256], FP8, tag="w2")

            st = [dict() for _ in range(BL)]

            ring_state = {"n": 0}

            def ring_zero(i, sv, h, eng=None):
                # pad ring of the 66x66 grid: top row (+1), bottom row, and
                # the left/right column pair of every row.  The ring stays
                # zero once written (signs only touch the interior), and the
                # pool has 6 slots, so only the first 6 tile allocations
                # need zeroing — later allocations land on already-zeroed
                # slots.  (Avoids false region deps stalling the signs.)
                if ring_state["n"] >= 12:       # 6 tiles x 2 halves
                    return
                ring_state["n"] += 1
                eng = eng or (nc.gpsimd if ring_eng == 'pool' else nc.vector)
                eng.memset(sv[:, h, 0:PW + 1], 0)
                eng.memset(sv[:, h, 65 * PW:HS], 0)
                lc = sv[:, h, 2 * PW - 1:2 * PW - 1 + 64 * PW].rearrange(
                    "p (r c) -> p r c", c=PW)
                eng.memset(lc[:, :, 0:2], 0)

            def sign_rows(i, sv, src, bias_col, h, r0, nr, eng):
                dst = sv[:, h, PW + r0 * PW:PW + (r0 + nr) * PW].rearrange(
                    "p (r c) -> p r c", c=PW)[:, :, 1:1 + W]
                s_src = src[:, r0 * W:(r0 + nr) * W].rearrange(
                    "p (y x) -> p y x", y=nr)
                if eng == 'v':
                    # s = (src >= -bias) - 0.5  -> {-0.5, +0.5} fp8
                    nc.vector.tensor_scalar(
                        dst, s_src, coef_t[:, bias_col + h:bias_col + h + 1],
                        0.5, op0=Alu.is_ge, op1=Alu.subtract)
                else:
                    # s = sign(src + bias) -> {-1, +1} fp8
                    nc.scalar.activation(
                        dst, s_src, mybir.ActivationFunctionType.Sign,
                        bias=coef_t[:, bias_col + h:bias_col + h + 1])

            def stage_A(i, first=False):
                # x DMA in row chunks; sign each chunk as it lands so conv1
                # can start early.  On the first image the weight DMAs are
                # interleaved after the chunks that gate the first matmuls
                # (w1 split per out-group) so x isn't stuck behind them.
                xs = []
                for h in range(2):
                    xb = px.tile([128, HW_], FP16, tag="x", name=f"x_{i}_{h}")
                    xs.append(xb)
                s = psn.tile([128, 2 * HS], FP8, tag="s", name=f"s_s1_{i}")
                sv = s[:, :].rearrange("p (h q) -> p h q", h=2, q=HS)
                st[i]["x"] = xs
                st[i]["s1"] = sv
                for h in range(2):
                    ring_zero(i, sv, h)
                chunks = [(0, 16), (16, 16), (32, 32)] if first else [(0, 32), (32, 32)]
                for chunk, (r0, nr) in enumerate(chunks):
                    for h in range(2):
                        # x chunks on the SP HWDGE queue; weights go via the
                        # Activation queue so they don't serialize behind x
                        # (gpsimd SWDGE is pathologically slow on HW)
                        dma_eng = nc.sync
                        dma_eng.dma_start(
                            out=xs[h][:, r0 * W:(r0 + nr) * W],
                            in_=x_ext[i, h * 128:(h + 1) * 128,
                                      r0:r0 + nr, :].rearrange(
                                          "c y x -> c (y x)"),
                        )
                    if first and chunk == 1:
                        # first matmuls only need w1's out-group 0
                        nc.scalar.dma_start(out=w1_t[:, :9 * 256],
                                            in_=w1_ext[:, :9 * 256])
                    for h in range(2):
                        sign_rows(i, sv, xs[h], 2, h, r0, nr, SIGN1_ENG)
                if first:
                    nc.scalar.dma_start(out=w1_t[:, 9 * 256:],
                                        in_=w1_ext[:, 9 * 256:])
                    nc.scalar.dma_start(out=w2_t[:, :], in_=w2_ext[:, :])

            def drain_B(i, g, y0, rows, ps):
                if probe in ('nodrain', 'nomm', 'justdma'):
                    return
                xs = st[i]["x"]
                n = rows * W
                t1 = pt.tile([128, 448], FP16, tag="t1")
                src = ps[:, 1:1 + rows * PW].rearrange(
                    "p (r c) -> p r c", c=PW)[:, :, 1:1 + W]
                nc.scalar.mul(
                    t1[:, :n].rearrange("p (r c) -> p r c", c=W),
                    src, coef_t[:, g:g + 1])
                xg = xs[g][:, y0 * W:y0 * W + n]
                nc.vector.tensor_add(xg, xg, t1[:, :n])
                # s2 half g is signed in 40/24-row chunks as xres completes,
                # so conv2's first 5-tile sweep (needs sign rows <= 36) can
                # start as early as possible
                if y0 == 35:
                    ring_zero(i, st[i]["s2"], g)
                    sign_rows(i, st[i]["s2"], xs[g], 4, g, 0, 40, SIGN2_ENG)
                elif y0 == 63:
                    sign_rows(i, st[i]["s2"], xs[g], 4, g, 40, 24, SIGN2_ENG)

            def prep_B(i):
                s2 = psn.tile([128, 2 * HS], FP8, tag="s", name=f"s_s2_{i}")
                st[i]["s2"] = s2[:, :].rearrange("p (h q) -> p h q", h=2, q=HS)

            def epilogue_part(i, x0, h0, nh):
                # fused prelu + pixel-unshuffle for output rows [h0, h0+nh)
                # of each of the 4 quadrants; one shared out DMA per chunk
                # (4 strided-in Prelu ops into one j-major tile).
                uv = x0[:, :].rearrange("p (h2 r1 w2 r2) -> p r1 r2 h2 w2",
                                        h2=32, r1=2, w2=32, r2=2)
                od = out_ext[i, :, :, :].rearrange("(c j) y x -> c j y x", j=4)
                y = py.tile([128, 2048], F32, tag="y")
                yv = y[:, :4 * nh * 32].rearrange("p (j a b) -> p j a b",
                                                  j=4, a=nh, b=32)
                for j in range(4):
                    r1, r2 = j >> 1, j & 1
                    nc.scalar.activation(
                        yv[:, j, :, :], uv[:, r1, r2, h0:h0 + nh, :],
                        mybir.ActivationFunctionType.Prelu,
                        alpha=coef_t[:, 8:9])
                    if use_b4:
                        nc.vector.tensor_scalar(
                            yv[:, j, :, :], yv[:, j, :, :],
                            coef_t[:, 9:10], None, op0=Alu.add)
                nc.sync.dma_start(out=od[:, :, h0:h0 + nh, :], in_=yv)

            def drain_D(i, y0, rows, ps):
                if probe in ('nodrain', 'noepi', 'nomm', 'justdma'):
                    return
                x0 = st[i]["x"][0]
                n = rows * W
                t2 = pt.tile([128, 448], FP16, tag="t1")
                src = ps[:, 1:1 + rows * PW].rearrange(
                    "p (r c) -> p r c", c=PW)[:, :, 1:1 + W]
                # t2 = psum * sc2 + bias3; engine selectable (ScalarE is
                # loaded in the conv2/epilogue region, but DVE PSUM reads
                # may be slower on HW).  The last tiles go to ScalarE
                # regardless: at the tail the DVE is the serial drain->add
                # chain, while ScalarE idles between epilogue batches.
                if t2_eng == 'v' and y0 < 49:
                    nc.vector.tensor_scalar(
                        t2[:, :n].rearrange("p (r c) -> p r c", c=W),
                        src, coef_t[:, 6:7], coef_t[:, 7:8],
                        op0=Alu.mult, op1=Alu.add)
                else:
                    nc.scalar.activation(
                        t2[:, :n].rearrange("p (r c) -> p r c", c=W),
                        src, Ident, bias=coef_t[:, 7:8],
                        scale=coef_t[:, 6:7])
                xb = x0[:, y0 * W:y0 * W + n]
                nc.vector.tensor_add(xb, xb, t2[:, :n])   # u = t2 + xres
                # epilogue in 3 chunks as rows complete (h2 <= (y0+5)/2)
                epi = {28: (0, 16), 49: (16, 11), 63: (27, 5)}.get(y0)
                if epi is not None:
                    epilogue_part(i, x0, epi[0], epi[1])

            def conv_unit(i, kind, g, tbi):
                # one PE work unit: 9-tap DoubleRow matmuls for one 5-tile
                # row sweep of conv<kind> for image i (out-group g),
                # followed by the tile drains.  5/5 sweeps (vs 4/4/2) load
                # each stationary weight once per 5 matmuls: 18 Ldweights
                # per conv-group instead of 27.
                sv = st[i]["s1"] if kind == 1 else st[i]["s2"]
                w_t = w1_t if kind == 1 else w2_t
                tb = (TILES[0:5], TILES[5:10])[tbi]
                pts = []
                for q, (y0, rows) in enumerate(tb):
                    pts.append(psum.tile([128, 512], F32, tag="ps",
                                         name=f"ps{kind}_{i}_{g}_{y0}"))
                for t in range(9):
                    if probe in ('nomm', 'justdma'):
                        break
                    ky, kx = t // 3, t % 3
                    col0 = (g * 9 + t) * 256 if kind == 1 else t * 256
                    wap = w_t[:, col0:col0 + 256].rearrange(
                        "p (h m) -> p h m", h=2)
                    for q, (y0, rows) in enumerate(tb):
                        n = rows * PW
                        off = PW * (y0 + ky)
                        nc.tensor.matmul(
                            pts[q][:, 2 - kx:2 - kx + n], wap,
                            sv[:, :, off:off + n],
                            start=(t == 0), stop=(t == 8),
                            perf_mode=DR,
                        )
                for q, (y0, rows) in enumerate(tb):
                    if kind == 1:
                        drain_B(i, g, y0, rows, pts[q])
                    else:
                        drain_D(i, y0, rows, pts[q])

            # Software-pipelined emission.  conv2(0) units are interleaved
            # into the conv1(1) window so image 0's drains/epilogue spread
            # over ~26us of matmuls instead of 7.4; conv2(1) follows with
            # only its last sweep's epilogue as the tail.
            for r in range(reps):
                stage_A(0, first=(r == 0))
                stage_A(1)
                prep_B(0)
                for g in (0, 1):
                    for tbi in (0, 1):
                        conv_unit(0, 1, g, tbi)
                prep_B(1)
                for (i, kind, g, tbi) in [
                    (1, 1, 0, 0), (1, 1, 0, 1), (0, 2, 0, 0),
                    (1, 1, 1, 0), (1, 1, 1, 1), (0, 2, 0, 1),
                ]:
                    conv_unit(i, kind, g, tbi)
                for tbi in (0, 1):
                    conv_unit(1, 2, 0, tbi)

    nc.compile()
    return nc


def _prep_weights(inputs):
    w1 = np.asarray(inputs["conv1_w"], np.float32)          # [256,256,3,3]
    w2 = np.asarray(inputs["conv2_w"], np.float32)          # [128,256,3,3]
    # DVE signs are +-0.5 (not +-1), so those conv scales carry an extra 2x
    f1 = 2.0 if SIGN1_ENG == 'v' else 1.0
    f2 = 2.0 if SIGN2_ENG == 'v' else 1.0
    sc1 = (f1 * np.abs(w1).mean(axis=(1, 2, 3))
           * float(np.asarray(inputs["kw1"]))
           * float(np.asarray(inputs["ka1"]))).astype(np.float32)   # [256]
    sc2 = (f2 * np.abs(w2).mean(axis=(1, 2, 3))
           * float(np.asarray(inputs["kw2"]))
           * float(np.asarray(inputs["ka2"]))).astype(np.float32)   # [128]

    # w1b[i, g, t, h, o] = sign(w1)[g*128+o, h*128+i, t//3, t%3]
    sgn1 = np.sign(w1).reshape(2, 128, 2, 128, 9)           # [g,o,h,i,t]
    w1b = np.ascontiguousarray(sgn1.transpose(3, 0, 4, 2, 1)
                               ).reshape(128, 18 * 256).astype(
                                   ml_dtypes.float8_e4m3fn)
    sgn2 = np.sign(w2).reshape(128, 2, 128, 9)              # [o,h,i,t]
    w2b = np.ascontiguousarray(sgn2.transpose(2, 3, 1, 0)
                               ).reshape(128, 9 * 256).astype(
                                   ml_dtypes.float8_e4m3fn)

    coef = np.zeros((128, 10), np.float32)
    coef[:, 0] = sc1[:128]
    coef[:, 1] = sc1[128:]
    b1 = np.asarray(inputs["bias1_"], np.float32).reshape(C)
    b2 = np.asarray(inputs["bias2_"], np.float32).reshape(C)
    if SIGN1_ENG == 'v':
        coef[:, 2] = -b1[:128]        # is_ge threshold = -bias
        coef[:, 3] = -b1[128:]
    else:
        coef[:, 2] = b1[:128]         # Sign activation bias = +bias
        coef[:, 3] = b1[128:]
    if SIGN2_ENG == 'v':
        coef[:, 4] = -b2[:128]
        coef[:, 5] = -b2[128:]
    else:
        coef[:, 4] = b2[:128]
        coef[:, 5] = b2[128:]
    coef[:, 6] = sc2
    coef[:, 7] = np.asarray(inputs["bias3"], np.float32).reshape(C // 2)
    coef[:, 8] = np.asarray(inputs["prelu2_w"], np.float32)
    coef[:, 9] = np.asarray(inputs["bias4"], np.float32).reshape(C // 2)
    return w1b, w2b, coef


def kernel(**inputs):
    return kernel_with_results(**inputs)[0]


def kernel_with_results(trace=False, **inputs):
    x = np.ascontiguousarray(np.asarray(inputs["x"], np.float32).astype(np.float16))
    w1b, w2b, coef = _prep_weights(inputs)
    use_b4 = bool(np.any(np.asarray(inputs["bias4"])))

    key = ("nc", use_b4)
    if key not in _CACHE:
        _CACHE[key] = build_nc(use_b4=use_b4)
    nc = _CACHE[key]

    in_maps = [
        {"x": x[i * BL:(i + 1) * BL], "w1": w1b, "w2": w2b, "coef": coef}
        for i in range(NCORES)
    ]
    res = run_bass_kernel_spmd(nc, in_maps, core_ids=list(range(NCORES)),
                               trace=trace)
    out = np.concatenate([res.results[i]["out"] for i in range(NCORES)], axis=0)
    return out, res
